# revision 33
# baseline (speedup 1.0000x reference)
"""Multi-head attention block (B=8, N=1024, D=1024, H=16, dh=64) on 8 TRN2 NeuronCores.

Strategy: data-parallel over batch (1 batch element per core). Per core, the whole
attention block runs out of SBUF in a feature-major ("transposed") dataflow that
avoids all on-device transposes:

  - qT/kT computed feature-major:  qkT[j, n]  = sum_d qkv_w[j, d] * x[n, d]   (lhsT=Wqk^T, rhs=x^T)
  - v computed token-major:        v[n, j]    = sum_d x[n, d] * Wv[j, d]      (lhsT=x^T, rhs=Wv^T)
  - scores transposed:             sT[k, q]   = sum_dh kT[dh, k] * qT[dh, q]  (K=64, row-packed head
                                   pairs -> the two matmuls run on concurrent PE row-tiles)
  - pattern:                       pT = exp(SCALE * sT)                        (ACT, PSUM->SBUF fp16)
  - zT + denominator fused:        [zT_h; den*64] = [v_h | 1*64]^T @ pT        (M=128: 64 v cols +
                                   64 ones cols -> den replicated on PSUM partitions 64..127, free)
  - normalize (no PE involvement): DVE reciprocal_approx_fast on den rows 64..127,
                                   DVE multiply (PSUM rows 0..63 x SBUF recip) -> zT fp16
  - output transposed:             outT[c, q] = sum_j proj_w[c, j] zT[j, q] + pb[c]

All matmul operands are fp16 (same 1 cyc/row PE rate as bf16 with 8x its mantissa
precision; every tensor here is O(1)-scaled so fp16 range is ample). fp16 (unlike
fp32r) also enables the PE's fast-weight-load path and halves DMA/SBUF traffic.
Biases: q/k bias via ACT per-partition bias on evacuation; v bias via a
host-pre-broadcast [128,1024] tile added by DVE on evacuation (no K=1 bias
matmuls); proj bias via ACT on evacuation.
"""
import os
import numpy as np
from contextlib import ExitStack

import concourse.bacc as bacc
import concourse.tile as tile
from concourse import mybir
from concourse.bass_utils import run_bass_kernel_spmd

f32 = mybir.dt.float32
f16 = mybir.dt.float16
AF = mybir.ActivationFunctionType

NB = 8          # batch / cores
N = 1024        # tokens
D = 1024        # d_model
H = 16          # heads
DH = 64         # head dim
SCALE = DH ** -0.5
NT = N // 128   # 8 token tiles
DT = D // 128   # 8 d tiles
HP = H // 2     # 8 head pairs

# Stashed results of the last run (for test harness introspection)
LAST_RESULTS = None
_NC_CACHE = None


def build_nc(loop_r=None):
    nc = bacc.Bacc("TRN2", target_bir_lowering=False, debug=False, enable_asserts=False)

    xp = nc.dram_tensor("xp", [128, DT * N], f16, kind="ExternalInput").ap()
    wqk = nc.dram_tensor("wqk", [128, HP * 2048], f16, kind="ExternalInput").ap()
    wv = nc.dram_tensor("wv", [128, 8192], f16, kind="ExternalInput").ap()
    pw = nc.dram_tensor("pw", [128, 8192], f16, kind="ExternalInput").ap()
    dbg = bool(os.environ.get("KDEBUG"))
    if dbg:
        d_zt = nc.dram_tensor("d_zt", [128, DT * N], f16, kind="ExternalOutput").ap()
        d_v0 = nc.dram_tensor("d_v0", [128, H * 128], f16, kind="ExternalOutput").ap()
        d_qa = nc.dram_tensor("d_qa", [128, N], f16, kind="ExternalOutput").ap()
        d_ka = nc.dram_tensor("d_ka", [128, N], f16, kind="ExternalOutput").ap()
    bqk = nc.dram_tensor("bqk", [128, 16], f32, kind="ExternalInput").ap()
    bvb = nc.dram_tensor("bvb", [128, 1024], f16, kind="ExternalInput").ap()
    pb = nc.dram_tensor("pb", [128, 8], f32, kind="ExternalInput").ap()
    onesd = nc.dram_tensor("onesd", [128, 1024], f16, kind="ExternalInput").ap()
    outT = nc.dram_tensor("outT", [D, N], f32, kind="ExternalOutput").ap()

    with tile.TileContext(nc) as tc, ExitStack() as ctx:
        const = ctx.enter_context(tc.tile_pool(name="const", bufs=1))
        xpool = ctx.enter_context(tc.tile_pool(name="xp", bufs=1))
        wvpool = ctx.enter_context(tc.tile_pool(name="wvp", bufs=1))
        vpool = ctx.enter_context(tc.tile_pool(name="vp", bufs=1))
        qkpool = ctx.enter_context(tc.tile_pool(name="qkp", bufs=4))
        ztpool = ctx.enter_context(tc.tile_pool(name="ztp", bufs=1))
        wqkpool = ctx.enter_context(tc.tile_pool(name="wqkp", bufs=2))
        pwpool = ctx.enter_context(tc.tile_pool(name="pwp", bufs=1))
        ptpool = ctx.enter_context(tc.tile_pool(name="ptp", bufs=4))
        mpool = ctx.enter_context(tc.tile_pool(name="mp", bufs=2))
        psum = ctx.enter_context(tc.tile_pool(name="ps", bufs=1, space="PSUM"))

        if loop_r is not None:
            ctx.enter_context(tc.For_i(
                0, loop_r, 1,
                hint_engines=(mybir.EngineType.PE, mybir.EngineType.Activation,
                              mybir.EngineType.DVE, mybir.EngineType.SP,
                              mybir.EngineType.Pool),
            ))

        # ---- constants (scalar/ACT HWDGE queue, parallel with x on SP) ----
        bqk_sb = const.tile([128, 16], f32, tag="bqk")
        nc.scalar.dma_start(bqk_sb, bqk)
        bvb_sb = const.tile([128, 1024], f16, tag="bvb")
        nc.scalar.dma_start(bvb_sb, bvb)
        pb_sb = const.tile([128, 8], f32, tag="pb")
        nc.scalar.dma_start(pb_sb, pb)

        # ---- persistent activations ----
        # Cold-start DMA is the first bottleneck: split x and wv across BOTH
        # HWDGE queues (SP + ACT) so they stream in parallel. After startup
        # the ACT queue must stay clear — DGE descriptor work between exps
        # would make ACT (which paces the attention phase) fall behind.
        xT = xpool.tile([128, DT * N], f16, tag="xT")        # [p, kt*N + n] = x[n, 128kt+p]
        wv_sb = wvpool.tile([128, 8192], f16, tag="wv")      # [p, jn*4096 + kt*512 + jj]
        for kt in range(DT):
            eng = nc.sync if kt % 2 == 0 else nc.scalar
            eng.dma_start(xT[:, kt * N: (kt + 1) * N], xp[:, kt * N: (kt + 1) * N])
        for c in range(4):
            eng = nc.sync if c % 2 == 0 else nc.scalar
            eng.dma_start(wv_sb[:, c * 2048: (c + 1) * 2048], wv[:, c * 2048: (c + 1) * 2048])

        # [p=token, 128h + c]; cols 128h+64..128h+127 == 1.0. The 64 ones
        # columns make the z-matmul emit the softmax denominator replicated on
        # PSUM partitions 64..127 (M is spatial on the PE, so this is free),
        # which lets the DVE normalize with no cross-partition data movement.
        # GpSimd memset of the whole tile (idle engine; a strided ones-DMA
        # would cost ~11K tiny descriptors on the startup-critical queues).
        v_sb = []
        for tt in range(NT):
            vt = vpool.tile([128, H * 128], f16, tag=f"v{tt}", name=f"v{tt}")
            nc.gpsimd.memset(vt, 1.0)
            v_sb.append(vt)

        zt = []                                              # [p=feature within tile, q]
        for jt in range(DT):
            zt.append(ztpool.tile([128, N], f16, tag=f"z{jt}", name=f"z{jt}"))

        # ---- phase V: v projection (token-major) ----
        for jn in range(2):
            for tt in range(NT):
                ps = psum.tile([128, 512], f32, tag="sps", name="ps_v", bufs=2)
                for kt in range(DT):
                    nc.tensor.matmul(
                        ps,
                        xT[:, kt * N + tt * 128: kt * N + tt * 128 + 128],
                        wv_sb[:, jn * 4096 + kt * 512: jn * 4096 + kt * 512 + 512],
                        start=(kt == 0), stop=(kt == DT - 1),
                    )
                dst = v_sb[tt][:, jn * 1024: jn * 1024 + 1024].rearrange(
                    "p (h c) -> p h c", c=128)[:, :, 0:64]
                with nc.allow_low_precision(reason="fp16 v storage"):
                    nc.vector.tensor_add(
                        dst,
                        ps.rearrange("p (h c) -> p h c", c=64),
                        bvb_sb[:, jn * 512: jn * 512 + 512].rearrange(
                            "p (h c) -> p h c", c=64),
                    )

        # ---- interleaved qkT projection + attention ----
        def make_qk_proj(hp):
            """Returns (qa, ka, generator). Generator emits 2 PE matmuls per step,
            16 steps total, with the ACT bias-evacuation attached to group ends."""
            wqk_t = wqkpool.tile([128, 2048], f16, tag="wqk", name=f"wqk{hp}")
            nc.sync.dma_start(wqk_t, wqk[:, hp * 2048: (hp + 1) * 2048])
            qa = qkpool.tile([128, N], f16, tag="qk", name=f"qa{hp}")
            ka = qkpool.tile([128, N], f16, tag="qk", name=f"ka{hp}")

            def gen():
                for dest, jt, which in ((qa, hp, 0), (ka, 8 + hp, 1)):
                    for qn in range(2):
                        ps = psum.tile([128, 512], f32, tag="mm", name="ps_qk", bufs=1)
                        for kt in range(DT):
                            base = kt * 256 + which * 128
                            nc.tensor.matmul(
                                ps,
                                wqk_t[:, base: base + 128],
                                xT[:, kt * N + qn * 512: kt * N + qn * 512 + 512],
                                start=(kt == 0), stop=(kt == DT - 1),
                            )
                            if kt % 2 == 1:
                                if kt == DT - 1:
                                    # DVE (not ACT) evacuation: keeps the ACT
                                    # engine free for the exp stream, which
                                    # paces the attention phase.
                                    with nc.allow_low_precision(reason="fp16 q/k"):
                                        nc.vector.tensor_scalar_add(
                                            dest[:, qn * 512: qn * 512 + 512], ps,
                                            bqk_sb[:, jt: jt + 1],
                                        )
                                yield
            return qa, ka, gen()

        qat, kat = {}, {}

        def emit_sps_exp(hp, qn, kt):
            sps = psum.tile([128, 1024], f32, tag="sps", name="sps", bufs=2)
            for h in range(2):
                off = h * 64
                nc.tensor.matmul(
                    sps[:, h * 512: h * 512 + 512],
                    kat[hp][off: off + 64, kt * 128: kt * 128 + 128],
                    qat[hp][off: off + 64, qn * 512: qn * 512 + 512],
                    start=True, stop=True,
                )
            pt = ptpool.tile([128, 1024], f16, tag="pt", name="pt")
            nc.scalar.activation(pt, sps, AF.Exp, scale=SCALE)
            return pt

        # Prefetch ALL out-projection weight tiles now (SP queue, overlapped
        # with the attention phase) — fetched lazily they arrive too late and
        # stall the final matmuls.
        pw_t = []
        for ct in range(DT):
            pwt = pwpool.tile([128, 1024], f16, tag=f"pw{ct}", name=f"pw{ct}")
            nc.sync.dma_start(pwt, pw[:, ct * 1024: (ct + 1) * 1024])
            pw_t.append(pwt)

        qat[0], kat[0], g = make_qk_proj(0)
        for _ in g:  # prologue: pair 0 projected un-interleaved
            pass

        # ---- flat attention pipeline over all (hp, qn, kt) steps ----
        # The score->exp prefetch runs 2 steps ahead CONTINUOUSLY across qn
        # and hp boundaries, so the ACT exp stream (which paces this phase)
        # never drains. Emission-order safety: the cross-hp prefetch at steps
        # 14/15 of pair hp only reads the qn0 halves of qa/ka[hp+1], whose
        # filler evacuations are emitted by step 13 (15 of 16 yields done).
        steps = [(hp, qn, kt) for hp in range(HP) for qn in range(2) for kt in range(NT)]
        pts = [emit_sps_exp(*steps[0]), emit_sps_exp(*steps[1])]
        filler = None
        zps = None
        for idx, (hp, qn, kt) in enumerate(steps):
            step_in_hp = idx % 16
            if step_in_hp == 0:
                filler = None
                if hp + 1 < HP:
                    qat[hp + 1], kat[hp + 1], filler = make_qk_proj(hp + 1)
            if kt == 0:
                zps = [psum.tile([128, 512], f32, tag="zps", name=f"zps{h}", bufs=3) for h in range(2)]
            pt = pts.pop(0)
            if idx + 2 < len(steps):
                pts.append(emit_sps_exp(*steps[idx + 2]))
            if filler is not None:
                nfill = 2 if step_in_hp == 0 else (0 if step_in_hp >= 14 else 1)
                for _ in range(nfill):
                    next(filler, None)
            for h in range(2):
                nc.tensor.matmul(
                    zps[h],
                    v_sb[kt][:, 128 * (2 * hp + h): 128 * (2 * hp + h) + 128],
                    pt[:, h * 512: h * 512 + 512],
                    start=(kt == 0), stop=(kt == NT - 1),
                )
            if kt == NT - 1:
                for h in range(2):
                    # zps rows 0..63 = z_raw, rows 64..127 = den (replicated).
                    # The custom-DVE reciprocal only works at base partition 0
                    # on HW, so copy the den rows down first (cross-base copy
                    # and PSUM x SBUF cross-base multiply are both fine).
                    zsd = mpool.tile([64, 512], f32, tag="zsd", name="zsd")
                    nc.vector.tensor_copy(zsd, zps[h][64: 128, :])
                    recip = mpool.tile([64, 512], f32, tag="recip", name="recip")
                    nc.vector.reciprocal_approx_fast(recip, zsd)
                    with nc.allow_low_precision(reason="fp16 rounding of attn out"):
                        nc.vector.tensor_mul(
                            zt[hp][h * 64: h * 64 + 64, qn * 512: qn * 512 + 512],
                            zps[h][0: 64, :], recip,
                        )
            if step_in_hp == 15 and filler is not None:
                for _ in filler:  # drain: the last yield carries the final evac
                    pass

        if dbg:
            nc.sync.dma_start(d_v0, v_sb[0])
            for jt in range(DT):
                nc.sync.dma_start(d_zt[:, jt * N: (jt + 1) * N], zt[jt])

        # ---- output projection (transposed) ----
        # NOTE: must be emitted entirely AFTER the attention loop: Tile
        # dependencies follow emission order, so reads of zt must come after
        # all writes.
        for ct in range(DT):
            for qn in range(2):
                ps = psum.tile([128, 512], f32, tag="sps", name="ps_o", bufs=2)
                for jt in range(DT):
                    nc.tensor.matmul(
                        ps,
                        pw_t[ct][:, jt * 128: jt * 128 + 128],
                        zt[jt][:, qn * 512: qn * 512 + 512],
                        start=(jt == 0), stop=(jt == DT - 1),
                    )
                ot = mpool.tile([128, 512], f32, tag="ot", name="ot")
                nc.scalar.activation(ot, ps, AF.Identity, bias=pb_sb[:, ct: ct + 1])
                # Alternate output DMAs across both HWDGE queues: on one queue
                # the last ~4MB of results serialize into a ~10us tail.
                eng = nc.sync if (ct * 2 + qn) % 2 == 0 else nc.scalar
                eng.dma_start(outT[ct * 128: ct * 128 + 128, qn * 512: qn * 512 + 512], ot)

    nc.compile()
    return nc


def prep_inputs(x, qkv_w, qkv_b, proj_w, proj_b):
    x = np.asarray(x, dtype=np.float32)
    qkv_w = np.asarray(qkv_w, dtype=np.float32)
    qkv_b = np.asarray(qkv_b, dtype=np.float32)
    proj_w = np.asarray(proj_w, dtype=np.float32)
    proj_b = np.asarray(proj_b, dtype=np.float32)
    f16 = np.float16

    # x^T packed: [b, p, kt*N + n] = x[b, n, 128kt+p]
    xp = (x.transpose(0, 2, 1).reshape(NB, DT, 128, N).transpose(0, 2, 1, 3)
          .reshape(NB, 128, DT * N)).astype(f16)

    wqkT = qkv_w[:2048, :].T                                  # [d, j']
    A4 = wqkT.reshape(DT, 128, 16, 128).transpose(1, 0, 2, 3)  # [p, kt, jt, jj]
    wqk_packed = (np.stack([A4[:, :, 0:8, :], A4[:, :, 8:16, :]], axis=3)
                  .transpose(0, 2, 1, 3, 4).reshape(128, HP * 2048)).astype(f16)

    wvT = qkv_w[2048:, :].T                                   # [d, j]
    wv_packed = wvT.reshape(DT, 128, 2, 512).transpose(1, 2, 0, 3).reshape(128, 8192).astype(f16)

    pwT = proj_w.T                                            # [j, c]
    pw_packed = pwT.reshape(DT, 128, DT, 128).transpose(1, 2, 0, 3).reshape(128, 8192).astype(f16)

    bqk_pt = np.ascontiguousarray(qkv_b[:2048].reshape(16, 128).T)
    bvb = np.broadcast_to(qkv_b[2048:].astype(f16).reshape(1, 1024), (128, 1024)).copy()
    pb_pt = np.ascontiguousarray(proj_b.reshape(8, 128).T)
    ones_np = np.ones((128, 1024), dtype=f16)

    shared = {
        "wqk": wqk_packed, "wv": wv_packed, "pw": pw_packed,
        "bqk": bqk_pt, "bvb": bvb, "pb": pb_pt, "onesd": ones_np,
    }
    return [{**shared, "xp": xp[b]} for b in range(NB)]


def kernel(x, qkv_w, qkv_b, proj_w, proj_b):
    global LAST_RESULTS, _NC_CACHE
    if _NC_CACHE is None:
        _NC_CACHE = build_nc()
    nc = _NC_CACHE
    in_maps = prep_inputs(x, qkv_w, qkv_b, proj_w, proj_b)
    res = run_bass_kernel_spmd(
        nc, in_maps, core_ids=list(range(NB)),
        trace=bool(os.environ.get("BASS_TRACE")),
    )
    LAST_RESULTS = res
    out = np.stack([np.ascontiguousarray(res.results[b]["outT"].T) for b in range(NB)])
    return out


# revision 37
# speedup vs baseline: 1.0568x; 1.0568x over previous
"""Multi-head attention block (B=8, N=1024, D=1024, H=16, dh=64) on 8 TRN2 NeuronCores.

Strategy: data-parallel over batch (1 batch element per core). Per core, the whole
attention block runs out of SBUF in a feature-major ("transposed") dataflow that
avoids all on-device transposes:

  - qT/kT computed feature-major:  qkT[j, n]  = sum_d qkv_w[j, d] * x[n, d]   (lhsT=Wqk^T, rhs=x^T)
  - v computed token-major:        v[n, j]    = sum_d x[n, d] * Wv[j, d]      (lhsT=x^T, rhs=Wv^T)
  - scores transposed:             sT[k, q]   = sum_dh kT[dh, k] * qT[dh, q]  (K=64, row-packed head
                                   pairs -> the two matmuls run on concurrent PE row-tiles)
  - pattern:                       pT = exp(SCALE * sT)                        (ACT, PSUM->SBUF fp16)
  - zT + denominator fused:        [zT_h; den*64] = [v_h | 1*64]^T @ pT        (M=128: 64 v cols +
                                   64 ones cols -> den replicated on PSUM partitions 64..127, free)
  - normalize (no PE involvement): DVE reciprocal_approx_fast on den rows 64..127,
                                   DVE multiply (PSUM rows 0..63 x SBUF recip) -> zT fp16
  - output transposed:             outT[c, q] = sum_j proj_w[c, j] zT[j, q] + pb[c]

All matmul operands are fp16 (same 1 cyc/row PE rate as bf16 with 8x its mantissa
precision; every tensor here is O(1)-scaled so fp16 range is ample). fp16 (unlike
fp32r) also enables the PE's fast-weight-load path and halves DMA/SBUF traffic.
Biases: q/k bias via ACT per-partition bias on evacuation; v bias via a
host-pre-broadcast [128,1024] tile added by DVE on evacuation (no K=1 bias
matmuls); proj bias via ACT on evacuation.
"""
import os
import numpy as np
from contextlib import ExitStack

import concourse.bacc as bacc
import concourse.tile as tile
from concourse import mybir
from concourse.bass_utils import run_bass_kernel_spmd

f32 = mybir.dt.float32
f16 = mybir.dt.float16
AF = mybir.ActivationFunctionType

NB = 8          # batch / cores
N = 1024        # tokens
D = 1024        # d_model
H = 16          # heads
DH = 64         # head dim
SCALE = DH ** -0.5
NT = N // 128   # 8 token tiles
DT = D // 128   # 8 d tiles
HP = H // 2     # 8 head pairs

# Stashed results of the last run (for test harness introspection)
LAST_RESULTS = None
_NC_CACHE = None


def build_nc(loop_r=None):
    nc = bacc.Bacc("TRN2", target_bir_lowering=False, debug=False, enable_asserts=False)

    xp = nc.dram_tensor("xp", [128, DT * N], f16, kind="ExternalInput").ap()
    wqk = nc.dram_tensor("wqk", [128, HP * 2048], f16, kind="ExternalInput").ap()
    wv = nc.dram_tensor("wv", [128, 8192], f16, kind="ExternalInput").ap()
    pw = nc.dram_tensor("pw", [128, 8192], f16, kind="ExternalInput").ap()
    dbg = bool(os.environ.get("KDEBUG"))
    if dbg:
        d_zt = nc.dram_tensor("d_zt", [128, DT * N], f16, kind="ExternalOutput").ap()
        d_v0 = nc.dram_tensor("d_v0", [128, H * 128], f16, kind="ExternalOutput").ap()
        d_qa = nc.dram_tensor("d_qa", [128, N], f16, kind="ExternalOutput").ap()
        d_ka = nc.dram_tensor("d_ka", [128, N], f16, kind="ExternalOutput").ap()
    bqk = nc.dram_tensor("bqk", [128, 16], f32, kind="ExternalInput").ap()
    bvb = nc.dram_tensor("bvb", [128, 1024], f16, kind="ExternalInput").ap()
    pb = nc.dram_tensor("pb", [128, 8], f32, kind="ExternalInput").ap()
    onesd = nc.dram_tensor("onesd", [128, 1024], f16, kind="ExternalInput").ap()
    outT = nc.dram_tensor("outT", [D, N], f32, kind="ExternalOutput").ap()

    with tile.TileContext(nc) as tc, ExitStack() as ctx:
        const = ctx.enter_context(tc.tile_pool(name="const", bufs=1))
        xpool = ctx.enter_context(tc.tile_pool(name="xp", bufs=1))
        wvpool = ctx.enter_context(tc.tile_pool(name="wvp", bufs=1))
        vpool = ctx.enter_context(tc.tile_pool(name="vp", bufs=1))
        qkpool = ctx.enter_context(tc.tile_pool(name="qkp", bufs=4))
        ztpool = ctx.enter_context(tc.tile_pool(name="ztp", bufs=1))
        wqkpool = ctx.enter_context(tc.tile_pool(name="wqkp", bufs=2))
        pwpool = ctx.enter_context(tc.tile_pool(name="pwp", bufs=1))
        ptpool = ctx.enter_context(tc.tile_pool(name="ptp", bufs=4))
        mpool = ctx.enter_context(tc.tile_pool(name="mp", bufs=2))
        psum = ctx.enter_context(tc.tile_pool(name="ps", bufs=1, space="PSUM"))

        if loop_r is not None:
            ctx.enter_context(tc.For_i(
                0, loop_r, 1,
                hint_engines=(mybir.EngineType.PE, mybir.EngineType.Activation,
                              mybir.EngineType.DVE, mybir.EngineType.SP,
                              mybir.EngineType.Pool),
            ))

        # ---- constants (scalar/ACT HWDGE queue, parallel with x on SP) ----
        bqk_sb = const.tile([128, 16], f32, tag="bqk")
        nc.scalar.dma_start(bqk_sb, bqk)
        bvb_sb = const.tile([128, 1024], f16, tag="bvb")
        nc.scalar.dma_start(bvb_sb, bvb)
        pb_sb = const.tile([128, 8], f32, tag="pb")
        nc.scalar.dma_start(pb_sb, pb)

        # ---- persistent activations ----
        # Cold-start DMA is the first bottleneck: split x and wv across BOTH
        # HWDGE queues (SP + ACT) so they stream in parallel. After startup
        # the ACT queue must stay clear — DGE descriptor work between exps
        # would make ACT (which paces the attention phase) fall behind.
        xT = xpool.tile([128, DT * N], f16, tag="xT")        # [p, kt*N + n] = x[n, 128kt+p]
        wv_sb = wvpool.tile([128, 8192], f16, tag="wv")      # [p, jn*4096 + kt*512 + jj]
        for kt in range(DT):
            eng = nc.sync if kt % 2 == 0 else nc.scalar
            eng.dma_start(xT[:, kt * N: (kt + 1) * N], xp[:, kt * N: (kt + 1) * N])
        for c in range(4):
            eng = nc.sync if c % 2 == 0 else nc.scalar
            eng.dma_start(wv_sb[:, c * 2048: (c + 1) * 2048], wv[:, c * 2048: (c + 1) * 2048])

        # [p=token, 128h + c]; cols 128h+0..63 == 1.0, cols 128h+64..127 = v.
        # The 64 ones columns make the z-matmul emit the softmax denominator
        # replicated on PSUM partitions 0..63 (M is spatial on the PE, so this
        # is free) — base 0, where the custom-DVE reciprocal works — with
        # z_raw on partitions 64..127. The DVE then normalizes straight out of
        # PSUM with no copies or cross-partition movement.
        # GpSimd memset of the whole tile (idle engine; a strided ones-DMA
        # would cost ~11K tiny descriptors on the startup-critical queues).
        v_sb = []
        for tt in range(NT):
            vt = vpool.tile([128, H * 128], f16, tag=f"v{tt}", name=f"v{tt}")
            nc.gpsimd.memset(vt, 1.0)
            v_sb.append(vt)

        zt = []                                              # [p=feature within tile, q]
        for jt in range(DT):
            zt.append(ztpool.tile([128, N], f16, tag=f"z{jt}", name=f"z{jt}"))

        # ---- phase V: v projection (token-major) ----
        for jn in range(2):
            for tt in range(NT):
                ps = psum.tile([128, 512], f32, tag="sps", name="ps_v", bufs=2)
                for kt in range(DT):
                    nc.tensor.matmul(
                        ps,
                        xT[:, kt * N + tt * 128: kt * N + tt * 128 + 128],
                        wv_sb[:, jn * 4096 + kt * 512: jn * 4096 + kt * 512 + 512],
                        start=(kt == 0), stop=(kt == DT - 1),
                    )
                dst = v_sb[tt][:, jn * 1024: jn * 1024 + 1024].rearrange(
                    "p (h c) -> p h c", c=128)[:, :, 64:128]
                with nc.allow_low_precision(reason="fp16 v storage"):
                    nc.vector.tensor_add(
                        dst,
                        ps.rearrange("p (h c) -> p h c", c=64),
                        bvb_sb[:, jn * 512: jn * 512 + 512].rearrange(
                            "p (h c) -> p h c", c=64),
                    )

        # ---- interleaved qkT projection + attention ----
        def make_qk_proj(hp):
            """Returns (qa, ka, generator). Generator emits 2 PE matmuls per step,
            16 steps total, with the ACT bias-evacuation attached to group ends."""
            wqk_t = wqkpool.tile([128, 2048], f16, tag="wqk", name=f"wqk{hp}")
            nc.sync.dma_start(wqk_t, wqk[:, hp * 2048: (hp + 1) * 2048])
            qa = qkpool.tile([128, N], f16, tag="qk", name=f"qa{hp}")
            ka = qkpool.tile([128, N], f16, tag="qk", name=f"ka{hp}")

            def gen():
                for dest, jt, which in ((qa, hp, 0), (ka, 8 + hp, 1)):
                    for qn in range(2):
                        ps = psum.tile([128, 512], f32, tag="mm", name="ps_qk", bufs=1)
                        for kt in range(DT):
                            base = kt * 256 + which * 128
                            nc.tensor.matmul(
                                ps,
                                wqk_t[:, base: base + 128],
                                xT[:, kt * N + qn * 512: kt * N + qn * 512 + 512],
                                start=(kt == 0), stop=(kt == DT - 1),
                            )
                            if kt % 2 == 1:
                                if kt == DT - 1:
                                    # DVE (not ACT) evacuation: keeps the ACT
                                    # engine free for the exp stream, which
                                    # paces the attention phase.
                                    with nc.allow_low_precision(reason="fp16 q/k"):
                                        nc.vector.tensor_scalar_add(
                                            dest[:, qn * 512: qn * 512 + 512], ps,
                                            bqk_sb[:, jt: jt + 1],
                                        )
                                yield
            return qa, ka, gen()

        qat, kat = {}, {}

        def emit_sps_exp(hp, qn, kt):
            sps = psum.tile([128, 1024], f32, tag="sps", name="sps", bufs=2)
            for h in range(2):
                off = h * 64
                nc.tensor.matmul(
                    sps[:, h * 512: h * 512 + 512],
                    kat[hp][off: off + 64, kt * 128: kt * 128 + 128],
                    qat[hp][off: off + 64, qn * 512: qn * 512 + 512],
                    start=True, stop=True,
                )
            pt = ptpool.tile([128, 1024], f16, tag="pt", name="pt")
            nc.scalar.activation(pt, sps, AF.Exp, scale=SCALE)
            return pt

        # Prefetch ALL out-projection weight tiles now (SP queue, overlapped
        # with the attention phase) — fetched lazily they arrive too late and
        # stall the final matmuls.
        pw_t = []
        for ct in range(DT):
            pwt = pwpool.tile([128, 1024], f16, tag=f"pw{ct}", name=f"pw{ct}")
            nc.sync.dma_start(pwt, pw[:, ct * 1024: (ct + 1) * 1024])
            pw_t.append(pwt)

        qat[0], kat[0], g = make_qk_proj(0)
        for _ in g:  # prologue: pair 0 projected un-interleaved
            pass

        # ---- flat attention pipeline over all (hp, qn, kt) steps ----
        # The score->exp prefetch runs 2 steps ahead CONTINUOUSLY across qn
        # and hp boundaries, so the ACT exp stream (which paces this phase)
        # never drains. Emission-order safety: the cross-hp prefetch at steps
        # 14/15 of pair hp only reads the qn0 halves of qa/ka[hp+1], whose
        # filler evacuations are emitted by step 13 (15 of 16 yields done).
        steps = [(hp, qn, kt) for hp in range(HP) for qn in range(2) for kt in range(NT)]
        pts = [emit_sps_exp(*steps[0]), emit_sps_exp(*steps[1])]
        filler = None
        zps = None
        for idx, (hp, qn, kt) in enumerate(steps):
            step_in_hp = idx % 16
            if step_in_hp == 0:
                filler = None
                if hp + 1 < HP:
                    qat[hp + 1], kat[hp + 1], filler = make_qk_proj(hp + 1)
            if kt == 0:
                zps = [psum.tile([128, 512], f32, tag="zps", name=f"zps{h}", bufs=3) for h in range(2)]
            pt = pts.pop(0)
            if idx + 2 < len(steps):
                pts.append(emit_sps_exp(*steps[idx + 2]))
            if filler is not None:
                # Concentrate filler matmuls right after each qn boundary
                # (steps 0 and 8): the extra PE work there covers the previous
                # qn's normalization-tail latency before its zps banks cycle.
                nfill = {0: 3, 6: 0, 7: 0, 8: 3, 14: 0, 15: 0}.get(step_in_hp, 1)
                for _ in range(nfill):
                    next(filler, None)
            for h in range(2):
                nc.tensor.matmul(
                    zps[h],
                    v_sb[kt][:, 128 * (2 * hp + h): 128 * (2 * hp + h) + 128],
                    pt[:, h * 512: h * 512 + 512],
                    start=(kt == 0), stop=(kt == NT - 1),
                )
                if kt == NT - 1:
                    # zps rows 0..63 = den (replicated), rows 64..127 = z_raw.
                    # recip reads PSUM at base 0 (custom-DVE ops only work
                    # there); the multiply reads PSUM at base 64 + SBUF base 0
                    # (mixed-space cross-base is fine). No copies.
                    recip = mpool.tile([64, 512], f32, tag="recip", name="recip")
                    nc.vector.reciprocal_approx_fast(recip, zps[h][0: 64, :])
                    with nc.allow_low_precision(reason="fp16 rounding of attn out"):
                        nc.vector.tensor_mul(
                            zt[hp][h * 64: h * 64 + 64, qn * 512: qn * 512 + 512],
                            zps[h][64: 128, :], recip,
                        )
            if step_in_hp == 15 and filler is not None:
                for _ in filler:  # drain: the last yield carries the final evac
                    pass

        if dbg:
            nc.sync.dma_start(d_v0, v_sb[0])
            for jt in range(DT):
                nc.sync.dma_start(d_zt[:, jt * N: (jt + 1) * N], zt[jt])

        # ---- output projection (transposed) ----
        # NOTE: must be emitted entirely AFTER the attention loop: Tile
        # dependencies follow emission order, so reads of zt must come after
        # all writes.
        for ct in range(DT):
            for qn in range(2):
                ps = psum.tile([128, 512], f32, tag="sps", name="ps_o", bufs=2)
                for jt in range(DT):
                    nc.tensor.matmul(
                        ps,
                        pw_t[ct][:, jt * 128: jt * 128 + 128],
                        zt[jt][:, qn * 512: qn * 512 + 512],
                        start=(jt == 0), stop=(jt == DT - 1),
                    )
                ot = mpool.tile([128, 512], f32, tag="ot", name="ot")
                nc.scalar.activation(ot, ps, AF.Identity, bias=pb_sb[:, ct: ct + 1])
                # Alternate output DMAs across both HWDGE queues: on one queue
                # the last ~4MB of results serialize into a ~10us tail.
                eng = nc.sync if (ct * 2 + qn) % 2 == 0 else nc.scalar
                eng.dma_start(outT[ct * 128: ct * 128 + 128, qn * 512: qn * 512 + 512], ot)

    nc.compile()
    return nc


def prep_inputs(x, qkv_w, qkv_b, proj_w, proj_b):
    x = np.asarray(x, dtype=np.float32)
    qkv_w = np.asarray(qkv_w, dtype=np.float32)
    qkv_b = np.asarray(qkv_b, dtype=np.float32)
    proj_w = np.asarray(proj_w, dtype=np.float32)
    proj_b = np.asarray(proj_b, dtype=np.float32)
    f16 = np.float16

    # x^T packed: [b, p, kt*N + n] = x[b, n, 128kt+p]
    xp = (x.transpose(0, 2, 1).reshape(NB, DT, 128, N).transpose(0, 2, 1, 3)
          .reshape(NB, 128, DT * N)).astype(f16)

    wqkT = qkv_w[:2048, :].T                                  # [d, j']
    A4 = wqkT.reshape(DT, 128, 16, 128).transpose(1, 0, 2, 3)  # [p, kt, jt, jj]
    wqk_packed = (np.stack([A4[:, :, 0:8, :], A4[:, :, 8:16, :]], axis=3)
                  .transpose(0, 2, 1, 3, 4).reshape(128, HP * 2048)).astype(f16)

    wvT = qkv_w[2048:, :].T                                   # [d, j]
    wv_packed = wvT.reshape(DT, 128, 2, 512).transpose(1, 2, 0, 3).reshape(128, 8192).astype(f16)

    pwT = proj_w.T                                            # [j, c]
    pw_packed = pwT.reshape(DT, 128, DT, 128).transpose(1, 2, 0, 3).reshape(128, 8192).astype(f16)

    bqk_pt = np.ascontiguousarray(qkv_b[:2048].reshape(16, 128).T)
    bvb = np.broadcast_to(qkv_b[2048:].astype(f16).reshape(1, 1024), (128, 1024)).copy()
    pb_pt = np.ascontiguousarray(proj_b.reshape(8, 128).T)
    ones_np = np.ones((128, 1024), dtype=f16)

    shared = {
        "wqk": wqk_packed, "wv": wv_packed, "pw": pw_packed,
        "bqk": bqk_pt, "bvb": bvb, "pb": pb_pt, "onesd": ones_np,
    }
    return [{**shared, "xp": xp[b]} for b in range(NB)]


def kernel(x, qkv_w, qkv_b, proj_w, proj_b):
    global LAST_RESULTS, _NC_CACHE
    if _NC_CACHE is None:
        _NC_CACHE = build_nc()
    nc = _NC_CACHE
    in_maps = prep_inputs(x, qkv_w, qkv_b, proj_w, proj_b)
    res = run_bass_kernel_spmd(
        nc, in_maps, core_ids=list(range(NB)),
        trace=bool(os.environ.get("BASS_TRACE")),
    )
    LAST_RESULTS = res
    out = np.stack([np.ascontiguousarray(res.results[b]["outT"].T) for b in range(NB)])
    return out


# revision 38
# speedup vs baseline: 1.0772x; 1.0193x over previous
"""Multi-head attention block (B=8, N=1024, D=1024, H=16, dh=64) on 8 TRN2 NeuronCores.

Strategy: data-parallel over batch (1 batch element per core). Per core, the whole
attention block runs out of SBUF in a feature-major ("transposed") dataflow that
avoids all on-device transposes:

  - qT/kT computed feature-major:  qkT[j, n]  = sum_d qkv_w[j, d] * x[n, d]   (lhsT=Wqk^T, rhs=x^T)
  - v computed token-major:        v[n, j]    = sum_d x[n, d] * Wv[j, d]      (lhsT=x^T, rhs=Wv^T)
  - scores transposed:             sT[k, q]   = sum_dh kT[dh, k] * qT[dh, q]  (K=64, row-packed head
                                   pairs -> the two matmuls run on concurrent PE row-tiles)
  - pattern:                       pT = exp(SCALE * sT)                        (ACT, PSUM->SBUF fp16)
  - zT + denominator fused:        [zT_h; den*64] = [v_h | 1*64]^T @ pT        (M=128: 64 v cols +
                                   64 ones cols -> den replicated on PSUM partitions 64..127, free)
  - normalize (no PE involvement): DVE reciprocal_approx_fast on den rows 64..127,
                                   DVE multiply (PSUM rows 0..63 x SBUF recip) -> zT fp16
  - output transposed:             outT[c, q] = sum_j proj_w[c, j] zT[j, q] + pb[c]

All matmul operands are fp16 (same 1 cyc/row PE rate as bf16 with 8x its mantissa
precision; every tensor here is O(1)-scaled so fp16 range is ample). fp16 (unlike
fp32r) also enables the PE's fast-weight-load path and halves DMA/SBUF traffic.
Biases: q/k bias via ACT per-partition bias on evacuation; v bias via a
host-pre-broadcast [128,1024] tile added by DVE on evacuation (no K=1 bias
matmuls); proj bias via ACT on evacuation.
"""
import os
import numpy as np
from contextlib import ExitStack

import concourse.bacc as bacc
import concourse.tile as tile
from concourse import mybir
from concourse.bass_utils import run_bass_kernel_spmd

f32 = mybir.dt.float32
f16 = mybir.dt.float16
AF = mybir.ActivationFunctionType

NB = 8          # batch / cores
N = 1024        # tokens
D = 1024        # d_model
H = 16          # heads
DH = 64         # head dim
SCALE = DH ** -0.5
NT = N // 128   # 8 token tiles
DT = D // 128   # 8 d tiles
HP = H // 2     # 8 head pairs

# Stashed results of the last run (for test harness introspection)
LAST_RESULTS = None
_NC_CACHE = None


def build_nc(loop_r=None):
    nc = bacc.Bacc("TRN2", target_bir_lowering=False, debug=False, enable_asserts=False)

    xp = nc.dram_tensor("xp", [128, DT * N], f16, kind="ExternalInput").ap()
    wqk = nc.dram_tensor("wqk", [128, HP * 2048], f16, kind="ExternalInput").ap()
    wv = nc.dram_tensor("wv", [128, 8192], f16, kind="ExternalInput").ap()
    pw = nc.dram_tensor("pw", [128, 8192], f16, kind="ExternalInput").ap()
    dbg = bool(os.environ.get("KDEBUG"))
    if dbg:
        d_zt = nc.dram_tensor("d_zt", [128, DT * N], f16, kind="ExternalOutput").ap()
        d_v0 = nc.dram_tensor("d_v0", [128, H * 128], f16, kind="ExternalOutput").ap()
        d_qa = nc.dram_tensor("d_qa", [128, N], f16, kind="ExternalOutput").ap()
        d_ka = nc.dram_tensor("d_ka", [128, N], f16, kind="ExternalOutput").ap()
    bqk = nc.dram_tensor("bqk", [128, 16], f32, kind="ExternalInput").ap()
    bvb = nc.dram_tensor("bvb", [128, 1024], f16, kind="ExternalInput").ap()
    pb = nc.dram_tensor("pb", [128, 8], f32, kind="ExternalInput").ap()
    onesd = nc.dram_tensor("onesd", [128, 1024], f16, kind="ExternalInput").ap()
    outT = nc.dram_tensor("outT", [D, N], f32, kind="ExternalOutput").ap()

    with tile.TileContext(nc) as tc, ExitStack() as ctx:
        const = ctx.enter_context(tc.tile_pool(name="const", bufs=1))
        xpool = ctx.enter_context(tc.tile_pool(name="xp", bufs=1))
        wvpool = ctx.enter_context(tc.tile_pool(name="wvp", bufs=1))
        vpool = ctx.enter_context(tc.tile_pool(name="vp", bufs=1))
        qkpool = ctx.enter_context(tc.tile_pool(name="qkp", bufs=4))
        ztpool = ctx.enter_context(tc.tile_pool(name="ztp", bufs=1))
        wqkpool = ctx.enter_context(tc.tile_pool(name="wqkp", bufs=2))
        pwpool = ctx.enter_context(tc.tile_pool(name="pwp", bufs=1))
        ptpool = ctx.enter_context(tc.tile_pool(name="ptp", bufs=4))
        mpool = ctx.enter_context(tc.tile_pool(name="mp", bufs=2))
        psum = ctx.enter_context(tc.tile_pool(name="ps", bufs=1, space="PSUM"))

        if loop_r is not None:
            ctx.enter_context(tc.For_i(
                0, loop_r, 1,
                hint_engines=(mybir.EngineType.PE, mybir.EngineType.Activation,
                              mybir.EngineType.DVE, mybir.EngineType.SP,
                              mybir.EngineType.Pool),
            ))

        # ---- persistent activations ----
        # Cold-start DMA is the first bottleneck: split x and wv across BOTH
        # HWDGE queues (SP + ACT) so they stream in parallel, ordered so the
        # first V-phase matmul group (which consumes wv jn=0 + x chunks in kt
        # order) is fed as early as possible. After startup the ACT queue must
        # stay clear — DGE descriptor work between exps would make ACT (which
        # co-paces the attention phase) fall behind.
        xT = xpool.tile([128, DT * N], f16, tag="xT")        # [p, kt*N + n] = x[n, 128kt+p]
        wv_sb = wvpool.tile([128, 8192], f16, tag="wv")      # [p, jn*4096 + kt*512 + jj]
        nc.sync.dma_start(wv_sb[:, 0: 2048], wv[:, 0: 2048])
        nc.scalar.dma_start(wv_sb[:, 2048: 4096], wv[:, 2048: 4096])
        for kt in range(DT):
            eng = nc.sync if kt % 2 == 0 else nc.scalar
            eng.dma_start(xT[:, kt * N: (kt + 1) * N], xp[:, kt * N: (kt + 1) * N])
        nc.sync.dma_start(wv_sb[:, 4096: 6144], wv[:, 4096: 6144])
        nc.scalar.dma_start(wv_sb[:, 6144: 8192], wv[:, 6144: 8192])

        # ---- constants (after the hot startup tensors) ----
        bvb_sb = const.tile([128, 1024], f16, tag="bvb")
        nc.scalar.dma_start(bvb_sb, bvb)
        bqk_sb = const.tile([128, 16], f32, tag="bqk")
        nc.scalar.dma_start(bqk_sb, bqk)
        pb_sb = const.tile([128, 8], f32, tag="pb")
        nc.scalar.dma_start(pb_sb, pb)

        # [p=token, 128h + c]; cols 128h+0..63 == 1.0, cols 128h+64..127 = v.
        # The 64 ones columns make the z-matmul emit the softmax denominator
        # replicated on PSUM partitions 0..63 (M is spatial on the PE, so this
        # is free) — base 0, where the custom-DVE reciprocal works — with
        # z_raw on partitions 64..127. The DVE then normalizes straight out of
        # PSUM with no copies or cross-partition movement.
        # GpSimd memset of the whole tile (idle engine; a strided ones-DMA
        # would cost ~11K tiny descriptors on the startup-critical queues).
        v_sb = []
        for tt in range(NT):
            vt = vpool.tile([128, H * 128], f16, tag=f"v{tt}", name=f"v{tt}")
            nc.gpsimd.memset(vt, 1.0)
            v_sb.append(vt)

        zt = []                                              # [p=feature within tile, q]
        for jt in range(DT):
            zt.append(ztpool.tile([128, N], f16, tag=f"z{jt}", name=f"z{jt}"))

        # ---- phase V: v projection (token-major) ----
        for jn in range(2):
            for tt in range(NT):
                ps = psum.tile([128, 512], f32, tag="sps", name="ps_v", bufs=2)
                for kt in range(DT):
                    nc.tensor.matmul(
                        ps,
                        xT[:, kt * N + tt * 128: kt * N + tt * 128 + 128],
                        wv_sb[:, jn * 4096 + kt * 512: jn * 4096 + kt * 512 + 512],
                        start=(kt == 0), stop=(kt == DT - 1),
                    )
                dst = v_sb[tt][:, jn * 1024: jn * 1024 + 1024].rearrange(
                    "p (h c) -> p h c", c=128)[:, :, 64:128]
                with nc.allow_low_precision(reason="fp16 v storage"):
                    nc.vector.tensor_add(
                        dst,
                        ps.rearrange("p (h c) -> p h c", c=64),
                        bvb_sb[:, jn * 512: jn * 512 + 512].rearrange(
                            "p (h c) -> p h c", c=64),
                    )

        # ---- interleaved qkT projection + attention ----
        def make_qk_proj(hp):
            """Returns (qa, ka, generator). Generator emits 2 PE matmuls per step,
            16 steps total, with the ACT bias-evacuation attached to group ends."""
            wqk_t = wqkpool.tile([128, 2048], f16, tag="wqk", name=f"wqk{hp}")
            nc.sync.dma_start(wqk_t, wqk[:, hp * 2048: (hp + 1) * 2048])
            qa = qkpool.tile([128, N], f16, tag="qk", name=f"qa{hp}")
            ka = qkpool.tile([128, N], f16, tag="qk", name=f"ka{hp}")

            def gen():
                for dest, jt, which in ((qa, hp, 0), (ka, 8 + hp, 1)):
                    for qn in range(2):
                        ps = psum.tile([128, 512], f32, tag="mm", name="ps_qk", bufs=1)
                        for kt in range(DT):
                            base = kt * 256 + which * 128
                            nc.tensor.matmul(
                                ps,
                                wqk_t[:, base: base + 128],
                                xT[:, kt * N + qn * 512: kt * N + qn * 512 + 512],
                                start=(kt == 0), stop=(kt == DT - 1),
                            )
                            if kt % 2 == 1:
                                if kt == DT - 1:
                                    # DVE (not ACT) evacuation: keeps the ACT
                                    # engine free for the exp stream, which
                                    # paces the attention phase.
                                    with nc.allow_low_precision(reason="fp16 q/k"):
                                        nc.vector.tensor_scalar_add(
                                            dest[:, qn * 512: qn * 512 + 512], ps,
                                            bqk_sb[:, jt: jt + 1],
                                        )
                                yield
            return qa, ka, gen()

        qat, kat = {}, {}

        def emit_sps_exp(hp, qn, kt):
            sps = psum.tile([128, 1024], f32, tag="sps", name="sps", bufs=2)
            for h in range(2):
                off = h * 64
                nc.tensor.matmul(
                    sps[:, h * 512: h * 512 + 512],
                    kat[hp][off: off + 64, kt * 128: kt * 128 + 128],
                    qat[hp][off: off + 64, qn * 512: qn * 512 + 512],
                    start=True, stop=True,
                )
            pt = ptpool.tile([128, 1024], f16, tag="pt", name="pt")
            nc.scalar.activation(pt, sps, AF.Exp, scale=SCALE)
            return pt

        # Prefetch ALL out-projection weight tiles now (SP queue, overlapped
        # with the attention phase) — fetched lazily they arrive too late and
        # stall the final matmuls.
        pw_t = []
        for ct in range(DT):
            pwt = pwpool.tile([128, 1024], f16, tag=f"pw{ct}", name=f"pw{ct}")
            nc.sync.dma_start(pwt, pw[:, ct * 1024: (ct + 1) * 1024])
            pw_t.append(pwt)

        qat[0], kat[0], g = make_qk_proj(0)
        for _ in g:  # prologue: pair 0 projected un-interleaved
            pass

        # ---- flat attention pipeline over all (hp, qn, kt) steps ----
        # The score->exp prefetch runs 2 steps ahead CONTINUOUSLY across qn
        # and hp boundaries, so the ACT exp stream (which paces this phase)
        # never drains. Emission-order safety: the cross-hp prefetch at steps
        # 14/15 of pair hp only reads the qn0 halves of qa/ka[hp+1], whose
        # filler evacuations are emitted by step 13 (15 of 16 yields done).
        steps = [(hp, qn, kt) for hp in range(HP) for qn in range(2) for kt in range(NT)]
        pts = [emit_sps_exp(*steps[0]), emit_sps_exp(*steps[1])]
        filler = None
        zps = None
        for idx, (hp, qn, kt) in enumerate(steps):
            step_in_hp = idx % 16
            if step_in_hp == 0:
                filler = None
                if hp + 1 < HP:
                    qat[hp + 1], kat[hp + 1], filler = make_qk_proj(hp + 1)
            if kt == 0:
                zps = [psum.tile([128, 512], f32, tag="zps", name=f"zps{h}", bufs=3) for h in range(2)]
            pt = pts.pop(0)
            if idx + 2 < len(steps):
                pts.append(emit_sps_exp(*steps[idx + 2]))
            if filler is not None:
                # Concentrate filler matmuls right after each qn boundary
                # (steps 0 and 8): the extra PE work there covers the previous
                # qn's normalization-tail latency before its zps banks cycle.
                nfill = {0: 3, 6: 0, 7: 0, 8: 3, 14: 0, 15: 0}.get(step_in_hp, 1)
                for _ in range(nfill):
                    next(filler, None)
            for h in range(2):
                nc.tensor.matmul(
                    zps[h],
                    v_sb[kt][:, 128 * (2 * hp + h): 128 * (2 * hp + h) + 128],
                    pt[:, h * 512: h * 512 + 512],
                    start=(kt == 0), stop=(kt == NT - 1),
                )
                if kt == NT - 1:
                    # zps rows 0..63 = den (replicated), rows 64..127 = z_raw.
                    # recip reads PSUM at base 0 (custom-DVE ops only work
                    # there); the multiply reads PSUM at base 64 + SBUF base 0
                    # (mixed-space cross-base is fine). No copies.
                    recip = mpool.tile([64, 512], f32, tag="recip", name="recip")
                    nc.vector.reciprocal_approx_fast(recip, zps[h][0: 64, :])
                    with nc.allow_low_precision(reason="fp16 rounding of attn out"):
                        nc.vector.tensor_mul(
                            zt[hp][h * 64: h * 64 + 64, qn * 512: qn * 512 + 512],
                            zps[h][64: 128, :], recip,
                        )
            if step_in_hp == 15 and filler is not None:
                for _ in filler:  # drain: the last yield carries the final evac
                    pass

        if dbg:
            nc.sync.dma_start(d_v0, v_sb[0])
            for jt in range(DT):
                nc.sync.dma_start(d_zt[:, jt * N: (jt + 1) * N], zt[jt])

        # ---- output projection (transposed) ----
        # NOTE: must be emitted entirely AFTER the attention loop: Tile
        # dependencies follow emission order, so reads of zt must come after
        # all writes.
        for ct in range(DT):
            for qn in range(2):
                ps = psum.tile([128, 512], f32, tag="sps", name="ps_o", bufs=2)
                for jt in range(DT):
                    nc.tensor.matmul(
                        ps,
                        pw_t[ct][:, jt * 128: jt * 128 + 128],
                        zt[jt][:, qn * 512: qn * 512 + 512],
                        start=(jt == 0), stop=(jt == DT - 1),
                    )
                ot = mpool.tile([128, 512], f32, tag="ot", name="ot")
                nc.scalar.activation(ot, ps, AF.Identity, bias=pb_sb[:, ct: ct + 1])
                # Alternate output DMAs across both HWDGE queues: on one queue
                # the last ~4MB of results serialize into a ~10us tail.
                eng = nc.sync if (ct * 2 + qn) % 2 == 0 else nc.scalar
                eng.dma_start(outT[ct * 128: ct * 128 + 128, qn * 512: qn * 512 + 512], ot)

    nc.compile()
    return nc


def prep_inputs(x, qkv_w, qkv_b, proj_w, proj_b):
    x = np.asarray(x, dtype=np.float32)
    qkv_w = np.asarray(qkv_w, dtype=np.float32)
    qkv_b = np.asarray(qkv_b, dtype=np.float32)
    proj_w = np.asarray(proj_w, dtype=np.float32)
    proj_b = np.asarray(proj_b, dtype=np.float32)
    f16 = np.float16

    # x^T packed: [b, p, kt*N + n] = x[b, n, 128kt+p]
    xp = (x.transpose(0, 2, 1).reshape(NB, DT, 128, N).transpose(0, 2, 1, 3)
          .reshape(NB, 128, DT * N)).astype(f16)

    wqkT = qkv_w[:2048, :].T                                  # [d, j']
    A4 = wqkT.reshape(DT, 128, 16, 128).transpose(1, 0, 2, 3)  # [p, kt, jt, jj]
    wqk_packed = (np.stack([A4[:, :, 0:8, :], A4[:, :, 8:16, :]], axis=3)
                  .transpose(0, 2, 1, 3, 4).reshape(128, HP * 2048)).astype(f16)

    wvT = qkv_w[2048:, :].T                                   # [d, j]
    wv_packed = wvT.reshape(DT, 128, 2, 512).transpose(1, 2, 0, 3).reshape(128, 8192).astype(f16)

    pwT = proj_w.T                                            # [j, c]
    pw_packed = pwT.reshape(DT, 128, DT, 128).transpose(1, 2, 0, 3).reshape(128, 8192).astype(f16)

    bqk_pt = np.ascontiguousarray(qkv_b[:2048].reshape(16, 128).T)
    bvb = np.broadcast_to(qkv_b[2048:].astype(f16).reshape(1, 1024), (128, 1024)).copy()
    pb_pt = np.ascontiguousarray(proj_b.reshape(8, 128).T)
    ones_np = np.ones((128, 1024), dtype=f16)

    shared = {
        "wqk": wqk_packed, "wv": wv_packed, "pw": pw_packed,
        "bqk": bqk_pt, "bvb": bvb, "pb": pb_pt, "onesd": ones_np,
    }
    return [{**shared, "xp": xp[b]} for b in range(NB)]


def kernel(x, qkv_w, qkv_b, proj_w, proj_b):
    global LAST_RESULTS, _NC_CACHE
    if _NC_CACHE is None:
        _NC_CACHE = build_nc()
    nc = _NC_CACHE
    in_maps = prep_inputs(x, qkv_w, qkv_b, proj_w, proj_b)
    res = run_bass_kernel_spmd(
        nc, in_maps, core_ids=list(range(NB)),
        trace=bool(os.environ.get("BASS_TRACE")),
    )
    LAST_RESULTS = res
    out = np.stack([np.ascontiguousarray(res.results[b]["outT"].T) for b in range(NB)])
    return out
